# revision 66
# baseline (speedup 1.0000x reference)
"""Bidirectional quantized RNN (fake-quant int8 weights/acts) on 8 trn2 cores.

Sequence-parallel sharding: each direction split into NCHUNK chunks of
L=seq/NCHUNK steps with W warmup steps (outputs discarded, chunk 0 exact);
core c handles direction c//4 and chunks (NCHUNK//4)*(c%4).. -> NCOL columns,
S=L+W steps.  The recurrent state is the bf16 tanh output th (validated:
bf16 rounding is washed out by the contraction).

Geometry tuned against the TimelineSim cost model (matmul cost =
out_free_rows * pe_cycle * cyc_per_row with fp8e4 DoubleRow = 0.5 cyc/row;
ACT op = free*0.833ns + ~185ns access + ~450ns round-trip latency to the
next dependent matmul):

* NCHUNK=32 -> 128 columns/core over 4 pipes x 32 cols, S=72 steps: the
  tanh->matmul round trip (~950ns) hides under ~1600ns/step of PE work,
  and each ACT op is big enough (free=128) to amortize its fixed cost.
* XB=4-step PSUM gate windows: 4 pipes x 2 windows x 1 bank = 8 banks.
* x-side matmuls in fp8e4 DoubleRow, EXACT at 0.75x bf16 row cost:
  j = j8 + dj (j8 = e4m3(j); dj = j - j8 is a small int <= 4, e4m3-exact);
  k_ri = 16*kh + kl (kh, kl in [-8,8], e4m3-exact).
    DR1(ic):      j8 @ 16kh_ic + dj @ 16kh_ic  (= j @ 16kh_ic exact)
    DR2(icA,icB): j8_A @ kl_A  + j8_B @ kl_B   (pairs two ic blocks)
  dropping only dj@kl (~0.002% of gate RMS).  J_MODE='round8' instead packs
  (j8@16kh + j8@kl) in ONE DR per ic (0.5x cost, ~1.5% gate noise).
* bias seeded by one fp8 DR matmul per (nck, window): bias_int = 2048A +
  128B + 8C + D/2 digit rows (e4m3-exact) against const rows 128,8,.5,1/32.
* recurrent matmuls stay EXACT bf16 (th moving, bf16(127*k_rh) stationary).
* x DMAs fetch two windows at a time (512B descriptors); one output DMA
  per 8-step slab.
"""
import numpy as np
import ml_dtypes
from contextlib import ExitStack

import concourse.bacc as bacc
import concourse.tile as tile
import concourse.mybir as mybir
from concourse.bass_utils import run_bass_kernel_spmd

SEQ, BATCH, IN, HID = 2048, 16, 512, 512
QMAX = np.float32(127.0)
F32 = mybir.dt.float32
BF16 = mybir.dt.bfloat16
FP8 = mybir.dt.float8e4
ACTF = mybir.ActivationFunctionType
DR = mybir.MatmulPerfMode.DoubleRow
E4 = ml_dtypes.float8_e4m3

import os
NCHUNK = 32     # chunks per direction
WARM = int(os.environ.get("K_WARM", "4"))   # cold-start warmup steps/chunk
XB = 4          # steps per PSUM gate window / x block
NCOL = (NCHUNK // 4) * BATCH  # columns per core = 128
NP = 4          # pipes
PC = NCOL // NP               # columns per pipe = 32
# x-term precision: 'exact12' (all ic exact), 'round10' (ic 0,1 use e4m3(j)
# directly: ~1.2% gate noise -> rel ~0.015), 'round8' (all rounded, ~0.019)
J_MODE = os.environ.get("K_JMODE", "round10")

_cache = {}


def _cache_key(seq):
    return (seq, 128 if seq >= 128 else 32, J_MODE, WARM, NCHUNK)


def _slab_schedule(S):
    assert S % 4 == 0 and S >= 16
    out = [8] * (S // 8)
    if S % 8:
        out.append(4)
    tail = out.pop()  # split the tail for a short final dependency chain
    out.extend([4, 2, 1, 1] if tail == 8 else [2, 1, 1])
    return out


def _build(S, N):
    """One SPMD program for all 8 cores. S = L+W local steps, N columns."""
    assert N == NCOL
    slabs = _slab_schedule(S)
    slab_start = []
    t0 = 0
    for ln in slabs:
        slab_start.append(t0)
        t0 += ln
    slab_of = {}
    for i, (st, ln) in enumerate(zip(slab_start, slabs)):
        for t in range(st, st + ln):
            slab_of[t] = (i, st, ln)
    nc = bacc.Bacc("TRN2")
    nxb = S // XB
    npair = (nxb + 1) // 2  # odd nxb: final pair is half-padded
    MV = XB * PC  # moving size per DR / per gate row block = 128
    # x packed [128, pipe, ic, pair, win, kind, 128]: per (p,ic,pair) the
    # (2 win x 2 kind x 128) block is contiguous -> 512B descriptors.
    x_p = nc.declare_dram_parameter("x", [128, NP, 4, npair, 2, 2, MV], FP8,
                                    isOutput=False)
    # x-side stationary: slot0 = kh/8, slot1 = kl/128 (k_ri/128 = slot0/16*..
    # exact-ic DR1 broadcasts slot0 on both k-tiles; rounded-ic DR1 uses both
    # slots; DR2 pairs slot1 across ic blocks.
    wk_p = nc.declare_dram_parameter("wk", [128, 4, 2, HID], FP8,
                                     isOutput=False)
    # recurrent weights bf16(127*k_rh/128), kc 0,1 only (kc 2,3 are fp8 DR)
    wrh_p = nc.declare_dram_parameter("wrh", [128, 2, HID], BF16,
                                      isOutput=False)
    # kc 2,3 recurrent weights W = 127*k_rh/128 = 16*Wh + Wl:
    # slot0 = 16*Wh (broadcast on k-tiles, pairs with (th8, dth) moving);
    # slot1 = e4m3(Wl) (kcD-paired, pairs with (th8_kc2, th8_kc3) moving)
    wm_p = nc.declare_dram_parameter("wm", [128, 2, 2, HID], FP8,
                                     isOutput=False)
    # bias digit rows [..., :512] (nck-major 128 chunks): rows 0..3 of tile0
    # = 16*digit; bias moving consts [..., 512:512+MV]: rows = 128,8,.5,1/32
    bwx_p = nc.declare_dram_parameter("bwx", [128, 2, 512 + MV], FP8,
                                      isOutput=False)
    cf_p = nc.declare_dram_parameter("cf", [128, 1], F32, isOutput=False)
    # th output, all pipes merged: [p, t, nck, col]
    out_p = nc.declare_dram_parameter("out", [128, S, 4, N], BF16,
                                      isOutput=True)

    with tile.TileContext(nc) as tc, ExitStack() as ctx:
        const = ctx.enter_context(tc.tile_pool(name="const", bufs=1))
        pJ = ctx.enter_context(tc.tile_pool(name="pJ", bufs=4))
        pM = ctx.enter_context(tc.tile_pool(name="pM", bufs=6))
        psG = [ctx.enter_context(tc.tile_pool(name=f"psG{p}", bufs=2,
                                              space="PSUM")) for p in range(NP)]

        j_pairs = [None] * npair
        g_tiles = [[None] * nxb for _ in range(NP)]

        def dma_pair(pr):
            jt = pJ.tile([128, NP, 4, 2, 2, MV], FP8, name="j", tag="j")
            nc.sync.dma_start(jt[:], x_p[:, :, :, pr, :, :, :])
            j_pairs[pr] = jt

        # Prologue DMA order tuned for the serial HWDGE/DMA-engines devices
        # (the step-0 recurrent matmuls are skipped -- zero initial state --
        # so the critical path is bias -> pipe-0 x -> x-weights, with the
        # recurrent weights needed only by step 1):
        #   sync: bwx, x-pair0-pipe0, wk, wrh, wm, x-pair0-rest, pair1
        #   gpsimd: cf (also delays nothing; tiny)
        cf_sb = const.tile([128, 1], F32, tag="cf")
        nc.gpsimd.dma_start(cf_sb[:], cf_p[:])
        bwx_sb = const.tile([128, 2, 512 + MV], FP8, tag="bwx")
        nc.sync.dma_start(bwx_sb[:], bwx_p[:])
        bw_sb = bwx_sb[:, :, 0:512]
        bx_sb = bwx_sb[:, :, 512:512 + MV]

        def load_weights():
            wk = const.tile([128, 4, 2, HID], FP8, tag="wk")
            nc.sync.dma_start(wk[:, 0:2, :, :], wk_p[:, 0:2, :, :])
            nc.sync.dma_start(wk[:, 2:4, :, :], wk_p[:, 2:4, :, :])
            wrh = const.tile([128, 2, HID], BF16, tag="wrh")
            nc.sync.dma_start(wrh[:], wrh_p[:])
            wm = const.tile([128, 2, 2, HID], FP8, tag="wm")
            nc.sync.dma_start(wm[:], wm_p[:])
            return wk, wrh, wm

        # Warm the ACT tanh table early.
        warm = const.tile([128, 1], F32, tag="warm")
        nc.scalar.activation(warm[:, 0:1], cf_sb[:, 0:1], ACTF.Tanh)

        jmm_queue = []  # deferred window-seeding matmuls, drained as PE filler

        def push_window(b, collect=None):
            jt, w = j_pairs[b // 2], b % 2
            for p in range(NP):
                if collect is not None:
                    jmm_queue = collect[p]
                g = psG[p].tile([128, 4, MV], F32, name="g", tag="g")
                g_tiles[p][b] = g
                # bias DR seeds; nck0 leads the 2KB PSUM bank (start marks
                # the whole bank pending-zero; later writes to pending bytes
                # overwrite).
                for nck in range(4):
                    jmm_queue.append((
                        g[:, nck, :], bw_sb[:, :, nck * 128:(nck + 1) * 128],
                        bx_sb[:], nck == 0))
                # DR1, exact ic:   (j8, dj) x bcast(kh/8)  == j @ kh/8
                #      rounded ic: (j8, j8) x (kh/8, kl/128) == j8 @ k/128
                n_round = {"exact12": 0, "round10": 2, "round8": 4}[J_MODE]
                for ic in range(4):
                    for nck in range(4):
                        lhsT = wk_sb[:, ic, :, nck * 128:(nck + 1) * 128]
                        if ic >= n_round:
                            lhsT = wk_sb[:, ic, 0:1,
                                         nck * 128:(nck + 1) * 128]
                            lhsT = lhsT.broadcast_to([128, 2, 128])
                        jmm_queue.append((
                            g[:, nck, :], lhsT, jt[:, p, ic, w, :, :],
                            False))
                # DR2 for the exact ic blocks: j8_A @ kl_A + j8_B @ kl_B
                if J_MODE == "exact12":
                    pairs = [(0, 2), (1, 3)]
                elif J_MODE == "round10":
                    pairs = [(2, 3)]  # ic 0,1 rounded; ic 2,3 exact
                else:
                    pairs = []
                for ica, icb in pairs:
                    st = icb - ica
                    for nck in range(4):
                        jmm_queue.append((
                            g[:, nck, :],
                            wk_sb[:, ica:icb + 1:st, 1,
                                  nck * 128:(nck + 1) * 128],
                            jt[:, p, ica:icb + 1:st, w, 0, :],
                            False))
            if bias_first:
                per = (len(jmm_queue) - mark) // NP  # instrs per pipe
                new = jmm_queue[mark:]
                jmm_queue[mark:] = (
                    [q for i, q in enumerate(new) if i % per < 4]
                    + [q for i, q in enumerate(new) if i % per >= 4])

        NFILL = NP * (4 + 16 + 4 * {"exact12": 2, "round10": 1,
                                    "round8": 0}[J_MODE])

        def emit_jmm(n):
            for _ in range(n):
                if not jmm_queue:
                    return
                out, lhsT, rhs, start = jmm_queue.pop(0)
                nc.tensor.matmul(out, lhsT, rhs, start=start, stop=False,
                                 perf_mode=DR, skip_group_check=True)

        # pair-0 pipe-0 x first (leads window-0 seeding), then the weights
        # (needed from step 1 -- step 0's recurrent term is zero and its
        # matmuls are skipped), then the rest of pair 0 and pair 1.
        jt0 = pJ.tile([128, NP, 4, 2, 2, MV], FP8, name="j", tag="j")
        nc.sync.dma_start(jt0[:, 0, :, :, :, :], x_p[:, 0, :, 0, :, :, :])
        wk_sb, wrh_t, wm_sb = load_weights()
        wrh_sb = [wrh_t[:, kc, :] for kc in range(2)]
        for p in range(1, NP):
            nc.sync.dma_start(jt0[:, p, :, :, :, :], x_p[:, p, :, 0, :, :, :])
        j_pairs[0] = jt0
        dma_pair(1)
        push_window(0, bias_first=True)
        emit_jmm(len(jmm_queue))
        push_window(1, bias_first=True)

        m_prev = [None] * NP
        tq_prev = [None] * NP
        prev_slot = [0] * NP
        mslab = tqslab = None

        for t in range(S):
            b, s = t // XB, t % XB
            sb_i, sb_st, sb_ln = slab_of[t]
            os = t - sb_st
            if s == 0 and b % 2 == 0 and 4 <= b + 4 < nxb:
                dma_pair((b + 4) // 2)
            if os == 0:
                mslab = pM.tile([128, sb_ln, 4, N], BF16, name="m", tag="m")
                tqslab = pM.tile([128, sb_ln, 2, 2, N], FP8, name="tq",
                                 tag="tq")
            for p in range(NP):
                gate = g_tiles[p][b]
                c0 = p * PC
                sl = prev_slot[p]
                # t == 0: zero initial state -> recurrent term vanishes;
                # skip the matmuls entirely (gate = bias + x terms).
                for kc in range(2 if t else 0):  # kc 0,1: exact bf16
                    for nck in range(4):
                        nc.tensor.matmul(
                            gate[:, nck, s * PC:(s + 1) * PC],
                            wrh_sb[kc][:, nck * 128:(nck + 1) * 128],
                            m_prev[p][:, sl, kc, c0:c0 + PC],
                            start=False, stop=False,
                            skip_group_check=True,
                        )
                # kc 2,3: fp8 DR, exact via th = th8 + dth split:
                # DR_a: th8 @ 16Wh + dth @ 16Wh  (= th @ 16Wh)
                for kcd in range(2 if t else 0):
                    for nck in range(4):
                        nc.tensor.matmul(
                            gate[:, nck, s * PC:(s + 1) * PC],
                            wm_sb[:, kcd, 0:1, nck * 128:(nck + 1) * 128]
                            .broadcast_to([128, 2, 128]),
                            tq_prev[p][:, sl, kcd, :, c0:c0 + PC],
                            start=False, stop=False,
                            perf_mode=DR, skip_group_check=True,
                        )
                # DR_b: th8_kc2 @ Wl_kc2 + th8_kc3 @ Wl_kc3
                for nck in range(4 if t else 0):
                    nc.tensor.matmul(
                        gate[:, nck, s * PC:(s + 1) * PC],
                        wm_sb[:, :, 1, nck * 128:(nck + 1) * 128],
                        tq_prev[p][:, sl, :, 0, c0:c0 + PC],
                        start=False, stop=(nck == 3),
                        perf_mode=DR, skip_group_check=True,
                    )
                emit_jmm((NFILL // XB) // NP)
                nc.scalar.activation(mslab[:, os, :, c0:c0 + PC],
                                     gate[:, :, s * PC:(s + 1) * PC],
                                     ACTF.Tanh, scale=cf_sb[:, 0:1])
                # split next-step DR state: th8 = fp8(th), dth = th - th8
                nc.vector.tensor_copy(tqslab[:, os, :, 0, c0:c0 + PC],
                                      mslab[:, os, 2:4, c0:c0 + PC])
                nc.vector.tensor_sub(tqslab[:, os, :, 1, c0:c0 + PC],
                                     mslab[:, os, 2:4, c0:c0 + PC],
                                     tqslab[:, os, :, 0, c0:c0 + PC])
                m_prev[p], tq_prev[p], prev_slot[p] = mslab, tqslab, os
            if s == XB - 1 and b + 2 < nxb:
                push_window(b + 2)
            if os == sb_ln - 1:
                eng = nc.scalar if sb_i == len(slabs) - 2 else nc.sync
                eng.dma_start(out_p[:, sb_st:sb_st + sb_ln, :, :], mslab[:])
    nc.compile()
    return nc


def _host_prep(inputs, seq):
    L = seq // NCHUNK
    S = L + WARM
    x = np.asarray(inputs["inputs"], np.float32)
    in_maps = []
    meta = []
    for d, (wri, wrh, b) in enumerate([
        (inputs["w_ri_f"], inputs["w_rh_f"], inputs["b_f"]),
        (inputs["w_ri_b"], inputs["w_rh_b"], inputs["b_b"]),
    ]):
        wri = np.asarray(wri, np.float32); wrh = np.asarray(wrh, np.float32)
        b = np.asarray(b, np.float32)
        threshold = np.float32(max(np.abs(wri).max(), np.abs(wrh).max()))
        s = np.float32(threshold / QMAX)
        k_ri = np.clip(np.round(wri / s), -QMAX, QMAX)
        k_rh = np.clip(np.round(wrh / s), -QMAX, QMAX)
        c_s = np.float32(np.float64(s) / 127.0)
        # Everything below works at gate' = gate/128 scale (so the kc 2,3
        # recurrent weights 127*k_rh/128 fit an exact-ish fp8 hi/lo split);
        # c_s compensates with a x128.
        # x-weight split: k_ri/128 = 16kh/128 + kl/128 = kh/8 + kl/128,
        # kh/kl in [-8,8] ints -> both power-2 scalings are e4m3-exact.
        kh = np.round(k_ri / 16.0)
        kl = k_ri - 16.0 * kh
        assert np.abs(kh).max() <= 8 and np.abs(kl).max() <= 8
        # [128, ic, 2slot, HID]: slot0 = kh/8, slot1 = kl/128
        wkh = (kh / 8.0).reshape(4, 128, HID).transpose(1, 0, 2)
        wklr = (kl / 128.0).reshape(4, 128, HID).transpose(1, 0, 2)
        wk = np.stack([wkh, wklr], axis=2)
        # kc 2,3 recurrent weights: W = 127*k_rh/128 = 16*Wh + Wl
        W = (127.0 * k_rh.astype(np.float64) / 128.0)
        Wh = np.round(W / 16.0)
        Wl = W - 16.0 * Wh
        assert np.abs(Wh).max() <= 8 and np.abs(Wl).max() <= 8.01
        WhT = (16.0 * Wh).reshape(4, 128, HID).transpose(1, 0, 2)  # [128,kc,H]
        WlT = Wl.reshape(4, 128, HID).transpose(1, 0, 2)
        wm = np.stack([WhT[:, 2:4, :], WlT[:, 2:4, :]], axis=2)  # [128,2,2,H]
        # bias digits at /128 scale: bias' = 16A + B + C/16 + D/256,
        # digit rows (16A, 16B, C, D/8) x const rows (1, 1/16, 1/16, 1/32)
        bias_p = b.astype(np.float64) / np.float64(c_s) / 128.0
        A = np.round(bias_p / 16.0); r = bias_p - 16.0 * A
        B = np.round(r); r -= B
        C = np.round(16.0 * r); r -= C / 16.0
        D = np.round(256.0 * r)
        assert max(np.abs(A).max(), np.abs(B).max(), np.abs(C).max(),
                   np.abs(D).max()) <= 8
        bwx = np.zeros((128, 2, 512 + XB * PC), np.float64)
        for r_i, dig in enumerate((16.0 * A, 16.0 * B, C, D / 8.0)):
            bwx[r_i, 0, 0:512] = dig
        for r_i, v in enumerate((1.0, 0.0625, 0.0625, 0.03125)):
            bwx[r_i, 0, 512:] = v
        cf = np.full((128, 1), np.float32(c_s * 128.0), np.float32)
        wrh2 = ((127.0 * k_rh / 128.0).reshape(4, 128, HID)
                .transpose(1, 0, 2))[:, 0:2, :]
        meta.append((np.ascontiguousarray(wk.astype(E4)),
                     np.ascontiguousarray(wrh2.astype(ml_dtypes.bfloat16)),
                     np.ascontiguousarray(wm.astype(E4)),
                     np.ascontiguousarray(bwx.astype(E4)), cf))
    xs = [x[:seq], x[:seq][::-1]]
    nxb = S // XB
    npair = (nxb + 1) // 2
    S_pad = npair * 2 * XB
    CPC = NCHUNK // 4  # chunks per core
    for core in range(8):
        d = core // 4
        wk, wrh2, wm, bwx, cf = meta[d]
        xd = xs[d]
        xT = np.empty((128, 4, S, NCOL), np.float32)
        for cl in range(CPC):
            q = CPC * (core % 4) + cl
            t0 = 0 if q == 0 else q * L - WARM
            blk = xd[t0:t0 + S]                     # [S, 16, 512]
            xT[:, :, :, cl * 16:(cl + 1) * 16] = (
                blk.transpose(2, 0, 1).reshape(4, 128, S, 16).transpose(1, 0, 2, 3))
        j = np.clip(np.round(127.0 * np.clip(xT, -1.0, 1.0)), -127.0, 127.0)
        j8 = j.astype(E4)
        dj = j - j8.astype(np.float32)
        assert np.abs(dj).max() <= 4
        if J_MODE == "round8":
            dj = j8.astype(np.float32)  # second moving slot = j8 again
        elif J_MODE == "round10":
            dj = dj.copy()
            dj[:, 0:2] = j8.astype(np.float32)[:, 0:2]

        if S_pad != S:
            pad = np.zeros((128, 4, S_pad - S, NCOL), np.float32)
            j8 = np.concatenate([j8.astype(np.float32), pad], axis=2).astype(E4)
            dj = np.concatenate([dj, pad], axis=2)

        # pack [128, pipe, ic, pair, win, kind, XB*PC]
        def pack(v):  # v [128, 4ic, S_pad, NCOL]
            v = v.reshape(128, 4, npair, 2, XB, NP, PC)
            return v.transpose(0, 5, 1, 2, 3, 4, 6)  # [128,NP,4,pair,win,XB,PC]
        xp = np.stack([pack(j8.astype(np.float32)), pack(dj)], axis=5)
        xp = np.ascontiguousarray(
            xp.reshape(128, NP, 4, npair, 2, 2, XB * PC).astype(E4))
        in_maps.append({"x": xp, "wk": wk, "wrh": wrh2, "wm": wm,
                        "bwx": bwx, "cf": cf})
    return in_maps


def _run(inputs, seq=SEQ, tb=None, trace=False):
    L = seq // NCHUNK
    S = L + WARM
    assert seq % NCHUNK == 0 and S % XB == 0
    key = _cache_key(seq)
    if key not in _cache:
        _cache[key] = _build(S, NCOL)
    nc = _cache[key]
    in_maps = _host_prep(inputs, seq)
    res = run_bass_kernel_spmd(nc, in_maps, core_ids=list(range(8)), trace=trace)
    out = np.empty((seq, BATCH, 2 * HID), np.float32)
    CPC = NCHUNK // 4
    for core in range(8):
        d = core // 4
        th = np.asarray(res.results[core]["out"], dtype=np.float32)
        m = np.clip(np.round(127.0 * th), -127.0, 127.0)
        h = m / np.float32(127.0)
        h = h.transpose(1, 3, 2, 0).reshape(S, NCOL, HID)  # [S, n, hid]
        for cl in range(CPC):
            q = CPC * (core % 4) + cl
            lo = 0 if q == 0 else WARM
            sl = h[lo:lo + L, cl * 16:(cl + 1) * 16, :]    # [L, 16, 512]
            if d == 0:
                out[q * L:(q + 1) * L, :, :HID] = sl
            else:
                out[seq - (q + 1) * L:seq - q * L, :, HID:] = sl[::-1]
    return out, res


def kernel(**inputs):
    out, _ = _run(inputs)
    return out
